# revision 13
# baseline (speedup 1.0000x reference)
"""GQA attention block (B=2, S=2048, D=1024, 16 q-heads / 4 kv-heads, RoPE,
softmax(QK^T/sqrt(D)) V, output projection) on 8 Trainium2 NeuronCores.

Sharding: core c = b*4 + g handles batch b and kv-group g (q-heads 4g..4g+3).
Each core computes its 4 heads' attention plus the corresponding 256 rows of
Wo, producing a partial (D, S) output; the host sums the 4 partials per batch.

v3 design (features on partitions, tokens on free):
  - K|V projection packed (one M=128 pass); Q projection per 128x1024 chunk.
  - RoPE on DVE: out = q*cos + shuffle(q*sin_pre_shuffled) using
    stream_shuffle for the pair swap; 1/sqrt(D) folded into the q tables.
  - Attention is software-pipelined: the PV matmul of k-tile j issues 2-3
    slots after its scores matmul, so the PE never waits on exp latency
    (keeps the HAM clock gate warm at 2.4 GHz).
  - exp: 12/16 k-tiles on ACT; 4/16 on DVE via a 3-op averaged-Schraudolph
    bit trick (~0.5% rel err; the softmax here is nearly flat so it washes).
  - Softmax denominator rides in PSUM row 64 of the PV accumulation (ones
    column in V^T); broadcast back via a ones-matmul into rows 64:128 of the
    same PSUM tile, reciprocal + scale on DVE.
  - Output projection: contraction 256 = 2 accumulating K=128 matmuls;
    bf16 partial outputs summed on host in f32.
"""

import sys
if "/opt/trn_rl_repo" not in sys.path:
    sys.path.insert(0, "/opt/trn_rl_repo")

import numpy as np
import ml_dtypes

B, S, D = 2, 2048, 1024
H, G, HD = 16, 4, 64
NCORES = 8
NKT = S // 128    # 16 k-token tiles
THETA = 10000.0
SCHRA_A = 2.0 ** 7 / np.log(2.0)   # 184.6627
SCHRA_B = 16249.0
SQRT2 = float(np.sqrt(2.0))

_compiled = None



def _build_program():
    import concourse.bass as bass
    import concourse.tile as tile
    import concourse.mybir as mybir
    from concourse import bacc
    from contextlib import ExitStack

    bf16 = mybir.dt.bfloat16
    f32 = mybir.dt.float32
    i16 = mybir.dt.int16
    EXP = mybir.ActivationFunctionType.Exp
    MUL = mybir.AluOpType.mult
    ADD = mybir.AluOpType.add

    nc = bacc.Bacc("TRN2", target_bir_lowering=False, debug=False,
                   num_devices=NCORES)

    def din(name, shape, dt=bf16):
        return nc.dram_tensor(name, shape, dt, kind="ExternalInput").ap()

    xT = din("xT", [D, S])
    wq = din("wq", [D, 256])
    wkv = din("wkv", [D, 128])
    wo = din("wo", [256, D])
    cq = din("cq", [256, S])
    sqp = din("sqp", [256, S])   # pre-shuffled (row pair-swapped) sin table
    ck = din("ck", [HD, S])
    sk = din("sk", [HD, S])
    outT = nc.dram_tensor("outT", [D, S], bf16, kind="ExternalOutput").ap()

    swap_mask = [i ^ 1 for i in range(32)]

    with tile.TileContext(nc) as tc, ExitStack() as ctx:
        pers = ctx.enter_context(tc.tile_pool(name="pers", bufs=1))

        def pt(name, shape, dt=bf16):
            return pers.tile(shape, dt, tag=name, name=name)

        xt_big = pt("xt_big", [128, 8, S])
        xt_s = [xt_big[:, i] for i in range(8)]
        wq_big = pt("wq_big", [128, 8, 256])
        wq_s = [wq_big[:, i] for i in range(8)]
        wkv_big = pt("wkv_big", [128, 8, 128])
        wkv_s = [wkv_big[:, i] for i in range(8)]
        wo_s = [pt(f"wo{i}", [128, D]) for i in range(2)]
        cq_s = [pt(f"cq{i}", [128, S]) for i in range(2)]
        sqp_s = [pt(f"sqp{i}", [128, S]) for i in range(2)]
        ck_s = pt("ck", [HD, S])
        sk_s = pt("sk", [HD, S])
        kvsb = pt("kvsb", [128, S])
        kdup = pt("kdup", [128, S])
        ksw = pt("ksw", [HD, S])
        kt1 = pt("kt1", [HD, S])
        qrope = [pt(f"qr{i}", [128, S]) for i in range(2)]
        v_big = pt("v_big", [128, NKT, 128])
        v_t = [v_big[:, i] for i in range(NKT)]
        ctxn = [pt(f"cx{i}", [128, S]) for i in range(2)]
        ones164 = pt("ones164", [1, HD])

        # constants / v_t padding init (no deps, runs during DMA)
        nc.vector.memset(ones164[:], 1.0)
        nc.vector.memset(v_big[:, :, HD:128], 0.0)
        for tt in range(NKT):
            nc.vector.memset(v_t[tt][:, HD:HD + 1], 1.0)

        # input DMA in consumption order, split across the two hwdge queues
        nc.sync.dma_start(wkv_big[:],
                          wkv.rearrange("(i p) c -> p i c", p=128))
        for i in range(8):
            nc.sync.dma_start(xt_s[i][:], xT[128 * i:128 * (i + 1), :])
        nc.scalar.dma_start(ck_s[:], ck[:])
        nc.scalar.dma_start(sk_s[:], sk[:])
        nc.scalar.dma_start(wq_big[:],
                            wq.rearrange("(i p) c -> p i c", p=128))
        for i in range(2):
            nc.scalar.dma_start(cq_s[i][:], cq[128 * i:128 * (i + 1), :])
            nc.scalar.dma_start(sqp_s[i][:], sqp[128 * i:128 * (i + 1), :])
        for i in range(2):
            nc.scalar.dma_start(wo_s[i][:], wo[128 * i:128 * (i + 1), :])

        ps = ctx.enter_context(tc.tile_pool(name="ps", bufs=2, space="PSUM"))
        ct = ctx.enter_context(tc.tile_pool(name="ct", bufs=2, space="PSUM"))
        sbp = ctx.enter_context(tc.tile_pool(name="sbp", bufs=5))
        sbq = ctx.enter_context(tc.tile_pool(name="sbq", bufs=2))
        sbo = ctx.enter_context(tc.tile_pool(name="sbo", bufs=3))
        sbs = ctx.enter_context(tc.tile_pool(name="sbs", bufs=2))

        # ------------- phase B: KV projection, K rope, V transpose -------
        for nch in range(2):
            pkv = ps.tile([128, 1024], f32, tag="ps", name="pkv")
            for h2 in range(2):
                s2 = slice(nch * 1024 + 512 * h2, nch * 1024 + 512 * (h2 + 1))
                for kt in range(8):
                    nc.tensor.matmul(pkv[:, 512 * h2:512 * (h2 + 1)],
                                     wkv_s[kt][:], xt_s[kt][:, s2],
                                     start=(kt == 0), stop=(kt == 7))
            nc.scalar.copy(kvsb[:, nch * 1024:(nch + 1) * 1024], pkv[:])

        nc.vector.stream_shuffle(ksw[:], kvsb[0:HD, :], swap_mask)
        nc.vector.tensor_mul(kt1[:], kvsb[0:HD, :], ck_s[:])
        nc.vector.tensor_mul(ksw[:], ksw[:], sk_s[:])
        nc.vector.tensor_add(kdup[0:HD, :], kt1[:], ksw[:])
        nc.sync.dma_start(kdup[HD:128, :], kdup[0:HD, :])
        nc.sync.dma_start_transpose(v_big[:, :, 0:HD], kvsb[HD:128, :])

        # ------------- Q projection + rope for one 1024-col chunk --------
        def qproj_chunk(mc, nch):
            sl = slice(nch * 1024, (nch + 1) * 1024)
            pq = ps.tile([128, 1024], f32, tag="ps", name="pq")
            for h2 in range(2):
                s2 = slice(nch * 1024 + 512 * h2, nch * 1024 + 512 * (h2 + 1))
                for kt in range(8):
                    nc.tensor.matmul(
                        pq[:, 512 * h2:512 * (h2 + 1)],
                        wq_s[kt][:, 128 * mc:128 * (mc + 1)],
                        xt_s[kt][:, s2], start=(kt == 0), stop=(kt == 7))
            qraw = sbq.tile([128, 1024], bf16, tag="qraw", name="qraw")
            nc.scalar.copy(qraw[:], pq[:])
            qt1 = sbq.tile([128, 1024], bf16, tag="qt1", name="qt1")
            nc.vector.tensor_mul(qt1[:], qraw[:], cq_s[mc][:, sl])
            qu = sbq.tile([128, 1024], bf16, tag="qu", name="qu")
            nc.vector.tensor_mul(qu[:], qraw[:], sqp_s[mc][:, sl])
            qsw = sbq.tile([128, 1024], bf16, tag="qsw", name="qsw")
            nc.vector.stream_shuffle(qsw[:], qu[:], swap_mask)
            nc.vector.tensor_add(qrope[mc][:, sl], qt1[:], qsw[:])

        qproj_chunk(0, 0)
        qproj_chunk(1, 0)

        # ------------- phase C: pipelined attention stream ---------------
        def attention_qc(qc, inject):
            """inject: list of (due_slot, fn) interleaved into the stream."""
            q0 = qc * 1024
            ctx_t = {}
            pv_done = {}
            pending = []     # (ready_slot, h, kt, pT)
            deferred = []    # (due_slot, seq, fn), kept sorted
            seq = [0]

            def push_deferred(due, fn):
                import bisect
                bisect.insort(deferred, (due, seq[0], fn))
                seq[0] += 1

            slot = [0]
            for due, fn in inject:
                push_deferred(due, fn)

            def emit_pv(h, kt, pT):
                cx = ctx_t[h]
                first = pv_done[h] == 0
                last = pv_done[h] == NKT - 1
                for h2 in range(2):
                    nc.tensor.matmul(cx[:, 512 * h2:512 * (h2 + 1)],
                                     v_t[kt][:],
                                     pT[:, 512 * h2:512 * (h2 + 1)],
                                     start=first, stop=last)
                pv_done[h] += 1
                if last:
                    schedule_norm(h)

            def schedule_norm(h):
                cx = ctx_t[h]
                g = slot[0]

                denr = sbs.tile([1, 1024], f32, tag="denr", name="denr")
                nc.scalar.copy(denr[:], cx[HD:HD + 1, :])
                rcp1_box = []

                def bcast():
                    rcp1 = sbs.tile([1, 1024], f32, tag="rcp1", name="rcp1")
                    nc.vector.reciprocal_approx_fast(rcp1[:], denr[:])
                    rcp = sbs.tile([HD, 1024], f32, tag="rcp", name="rcp")
                    nc.gpsimd.partition_broadcast(rcp[:], rcp1[:])
                    rcp1_box.append(rcp)

                def finish():
                    hb = HD * (h % 2)
                    nc.vector.tensor_mul(
                        ctxn[h // 2][hb:hb + HD, q0:q0 + 1024],
                        cx[0:HD, :], rcp1_box[0][:])

                push_deferred(g + 1, bcast)
                push_deferred(g + 3, finish)

            for h in range(4):
                ctx_t[h] = None
                pv_done[h] = 0
                mcq, hb = h // 2, HD * (h % 2)
                qt = qrope[mcq]
                for kt in range(NKT):
                    g = slot[0]
                    # scores for (h, kt)
                    s = ps.tile([128, 1024], f32, tag="ps", name="s")
                    for h2 in range(2):
                        nc.tensor.matmul(
                            s[:, 512 * h2:512 * (h2 + 1)],
                            kdup[hb:hb + HD, 128 * kt:128 * (kt + 1)],
                            qt[hb:hb + HD, q0 + 512 * h2:q0 + 512 * (h2 + 1)],
                            start=True, stop=True)
                    pT = sbp.tile([128, 1024], bf16, tag="pT", name="pT")
                    if kt in (5, 13):
                        v1 = sbp.tile([128, 1024], bf16, tag="v1", name="v1",
                                      bufs=2)
                        nc.vector.tensor_scalar(
                            v1[:].bitcast(i16), s[:], SCHRA_A,
                            SCHRA_B - 192.0, MUL, ADD)
                        v2 = sbp.tile([128, 1024], bf16, tag="v2", name="v2",
                                      bufs=2)
                        nc.vector.tensor_scalar(
                            v2[:].bitcast(i16), v1[:].bitcast(i16), 64.0,
                            None, ADD)
                        nc.vector.scalar_tensor_tensor(
                            pT[:], v1[:], SQRT2, v2[:], MUL, ADD)
                        ready = g + 4
                    else:
                        nc.scalar.activation(pT[:], s[:], EXP)
                        ready = g + 2
                    if ctx_t[h] is None:
                        ctx_t[h] = ct.tile([128, 1024], f32, tag="ct",
                                           name="cx")
                    pending.append((ready, h, kt, pT))
                    slot[0] += 1
                    # emit due PVs / deferred work
                    while pending and pending[0][0] <= slot[0]:
                        _, ph, pkt, ppT = pending.pop(0)
                        emit_pv(ph, pkt, ppT)
                    while deferred and deferred[0][0] <= slot[0]:
                        deferred.pop(0)[2]()

            # flush
            while pending:
                _, ph, pkt, ppT = pending.pop(0)
                emit_pv(ph, pkt, ppT)
                slot[0] += 1
                while deferred and deferred[0][0] <= slot[0]:
                    deferred.pop(0)[2]()
            slot[0] += 4
            while deferred:
                deferred.pop(0)[2]()

        def d_chunk(qc, mc):
            q0 = qc * 1024
            dp = ps.tile([128, 1024], f32, tag="ps", name="dp")
            for h2 in range(2):
                s2 = slice(q0 + 512 * h2, q0 + 512 * (h2 + 1))
                nc.tensor.matmul(dp[:, 512 * h2:512 * (h2 + 1)],
                                 wo_s[0][:, 128 * mc:128 * (mc + 1)],
                                 ctxn[0][:, s2], start=True, stop=False)
                nc.tensor.matmul(dp[:, 512 * h2:512 * (h2 + 1)],
                                 wo_s[1][:, 128 * mc:128 * (mc + 1)],
                                 ctxn[1][:, s2], start=False, stop=True)
            ob = sbo.tile([128, 1024], bf16, tag="ob", name="ob")
            if mc % 2 == 0:
                nc.vector.tensor_copy(ob[:], dp[:])
            else:
                nc.scalar.copy(ob[:], dp[:])
            nc.sync.dma_start(outT[128 * mc:128 * (mc + 1), q0:q0 + 1024],
                              ob[:])

        # qc0: q-projection for the second column block rides at head
        # boundaries; D(qc0) is interleaved into C(qc1)'s early slots.
        attention_qc(0, [(24, lambda: qproj_chunk(0, 1)),
                         (40, lambda: qproj_chunk(1, 1))])
        attention_qc(1, [(4 + 2 * mc, (lambda m: lambda: d_chunk(0, m))(mc))
                        for mc in range(8)])
        for mc in range(8):
            d_chunk(1, mc)

    nc.compile()
    return nc


def _host_inputs(x, Wq, Wk, Wv, Wo):
    """Build the 8 per-core input maps."""
    bf = ml_dtypes.bfloat16
    inv = 1.0 / (THETA ** (np.arange(0, D, 2, dtype=np.float64) / D))
    t = np.arange(S, dtype=np.float64)
    sgn256 = np.where(np.arange(256) % 2 == 0, -1.0, 1.0)
    sgn64 = sgn256[:HD]
    INVSQ = 1.0 / 32.0   # 1/sqrt(D), folded into the q rope tables
    swap = np.arange(256) ^ 1

    angk = t[None, :] * inv[np.arange(HD) // 2][:, None]
    ck = np.cos(angk).astype(bf)
    sk = (sgn64[:, None] * np.sin(angk)).astype(bf)

    in_maps = []
    for c in range(NCORES):
        b, g = divmod(c, G)
        fq = inv[128 * g + np.arange(256) // 2]
        angq = t[None, :] * fq[:, None]
        sq = INVSQ * sgn256[:, None] * np.sin(angq)
        in_maps.append({
            "xT": np.ascontiguousarray(x[b].T).astype(bf),
            "wq": np.ascontiguousarray(Wq[:, 256 * g:256 * (g + 1)]).astype(bf),
            "wkv": np.ascontiguousarray(np.concatenate(
                [Wk[:, HD * g:HD * (g + 1)],
                 Wv[:, HD * g:HD * (g + 1)]], axis=1)).astype(bf),
            "wo": np.ascontiguousarray(Wo[256 * g:256 * (g + 1), :]).astype(bf),
            "cq": (INVSQ * np.cos(angq)).astype(bf),
            "sqp": np.ascontiguousarray(sq[swap]).astype(bf),
            "ck": ck, "sk": sk,
        })
    return in_maps


def _run(in_maps, trace=False, tmpdir=None):
    global _compiled
    from concourse.bass_utils import run_bass_kernel_spmd
    if _compiled is None:
        _compiled = _build_program()
    return run_bass_kernel_spmd(_compiled, in_maps, list(range(NCORES)),
                                trace=trace, tmpdir=tmpdir)


def kernel(x, Wq, Wk, Wv, Wo, _trace=False, _tmpdir=None):
    x = np.asarray(x, np.float32)
    in_maps = _host_inputs(x, np.asarray(Wq, np.float32),
                           np.asarray(Wk, np.float32),
                           np.asarray(Wv, np.float32),
                           np.asarray(Wo, np.float32))
    res = _run(in_maps, trace=_trace, tmpdir=_tmpdir)
    out = np.zeros((B, S, D), np.float32)
    for c in range(NCORES):
        b = c // G
        out[b] += res.results[c]["outT"].T.astype(np.float32)
    kernel.last_results = res
    return out


# revision 14
# speedup vs baseline: 1.0670x; 1.0670x over previous
"""GQA attention block (B=2, S=2048, D=1024, 16 q-heads / 4 kv-heads, RoPE,
softmax(QK^T/sqrt(D)) V, output projection) on 8 Trainium2 NeuronCores.

Sharding: core c = b*4 + g handles batch b and kv-group g (q-heads 4g..4g+3).
Each core computes its 4 heads' attention plus the corresponding 256 rows of
Wo, producing a partial (D, S) output; the host sums the 4 partials per batch.

v3 design (features on partitions, tokens on free):
  - K|V projection packed (one M=128 pass); Q projection per 128x1024 chunk.
  - RoPE on DVE: out = q*cos + shuffle(q*sin_pre_shuffled) using
    stream_shuffle for the pair swap; 1/sqrt(D) folded into the q tables.
  - Attention is software-pipelined: the PV matmul of k-tile j issues 2-3
    slots after its scores matmul, so the PE never waits on exp latency
    (keeps the HAM clock gate warm at 2.4 GHz).
  - exp: 12/16 k-tiles on ACT; 4/16 on DVE via a 3-op averaged-Schraudolph
    bit trick (~0.5% rel err; the softmax here is nearly flat so it washes).
  - Softmax denominator rides in PSUM row 64 of the PV accumulation (ones
    column in V^T); broadcast back via a ones-matmul into rows 64:128 of the
    same PSUM tile, reciprocal + scale on DVE.
  - Output projection: contraction 256 = 2 accumulating K=128 matmuls;
    bf16 partial outputs summed on host in f32.
"""

import sys
if "/opt/trn_rl_repo" not in sys.path:
    sys.path.insert(0, "/opt/trn_rl_repo")

import numpy as np
import ml_dtypes

B, S, D = 2, 2048, 1024
H, G, HD = 16, 4, 64
NCORES = 8
NKT = S // 128    # 16 k-token tiles
THETA = 10000.0
SCHRA_A = 2.0 ** 7 / np.log(2.0)   # 184.6627
SCHRA_B = 16249.0
SQRT2 = float(np.sqrt(2.0))

_compiled = None



def _build_program():
    import concourse.bass as bass
    import concourse.tile as tile
    import concourse.mybir as mybir
    from concourse import bacc
    from contextlib import ExitStack

    bf16 = mybir.dt.bfloat16
    f32 = mybir.dt.float32
    i16 = mybir.dt.int16
    f8 = mybir.dt.float8e4
    EXP = mybir.ActivationFunctionType.Exp
    MUL = mybir.AluOpType.mult
    ADD = mybir.AluOpType.add

    nc = bacc.Bacc("TRN2", target_bir_lowering=False, debug=False,
                   num_devices=NCORES)

    def din(name, shape, dt=bf16):
        return nc.dram_tensor(name, shape, dt, kind="ExternalInput").ap()

    xT = din("xT", [D, S])
    wq8d = din("wq8d", [D, 256], mybir.dt.float8e4)
    xq8d = din("xq8d", [D, S], mybir.dt.float8e4)
    wkv = din("wkv", [D, 128])
    wo = din("wo", [256, D])
    cq = din("cq", [256, S])
    sqp = din("sqp", [256, S])   # pre-shuffled (row pair-swapped) sin table
    ck = din("ck", [HD, S])
    sk = din("sk", [HD, S])
    outT = nc.dram_tensor("outT", [D, S], bf16, kind="ExternalOutput").ap()

    swap_mask = [i ^ 1 for i in range(32)]

    with tile.TileContext(nc) as tc, ExitStack() as ctx:
        pers = ctx.enter_context(tc.tile_pool(name="pers", bufs=1))

        def pt(name, shape, dt=bf16):
            return pers.tile(shape, dt, tag=name, name=name)

        xt_big = pt("xt_big", [128, 8, S])
        xt_s = [xt_big[:, i] for i in range(8)]
        wq8 = pt("wq8", [128, 8, 256], f8)
        xq8 = pt("xq8", [128, 8, S], f8)
        wkv_big = pt("wkv_big", [128, 8, 128])
        wkv_s = [wkv_big[:, i] for i in range(8)]
        wo_s = [pt(f"wo{i}", [128, D]) for i in range(2)]
        cq_s = [pt(f"cq{i}", [128, S]) for i in range(2)]
        sqp_s = [pt(f"sqp{i}", [128, S]) for i in range(2)]
        ck_s = pt("ck", [HD, S])
        sk_s = pt("sk", [HD, S])
        kvsb = pt("kvsb", [128, S])
        kdup = pt("kdup", [128, S])
        ksw = pt("ksw", [HD, S])
        kt1 = pt("kt1", [HD, S])
        qrope = [pt(f"qr{i}", [128, S]) for i in range(2)]
        v_big = pt("v_big", [128, NKT, 128])
        v_t = [v_big[:, i] for i in range(NKT)]
        ctxn = [pt(f"cx{i}", [128, S]) for i in range(2)]
        ones164 = pt("ones164", [1, HD])

        # constants / v_t padding init (no deps, runs during DMA)
        nc.vector.memset(ones164[:], 1.0)
        nc.vector.memset(v_big[:, :, HD:128], 0.0)
        for tt in range(NKT):
            nc.vector.memset(v_t[tt][:, HD:HD + 1], 1.0)

        # input DMA in consumption order, split across the two hwdge queues
        nc.sync.dma_start(wkv_big[:],
                          wkv.rearrange("(i p) c -> p i c", p=128))
        for i in range(8):
            nc.sync.dma_start(xt_s[i][:], xT[128 * i:128 * (i + 1), :])
        nc.sync.dma_start(ck_s[:], ck[:])
        nc.sync.dma_start(sk_s[:], sk[:])
        nc.sync.dma_start(wq8[:], wq8d.rearrange("(i p) c -> p i c", p=128))
        nc.sync.dma_start(xq8[:], xq8d.rearrange("(i p) c -> p i c", p=128))
        for i in range(2):
            nc.sync.dma_start(cq_s[i][:], cq[128 * i:128 * (i + 1), :])
            nc.sync.dma_start(sqp_s[i][:], sqp[128 * i:128 * (i + 1), :])
        for i in range(2):
            nc.sync.dma_start(wo_s[i][:], wo[128 * i:128 * (i + 1), :])

        ps = ctx.enter_context(tc.tile_pool(name="ps", bufs=2, space="PSUM"))
        ct = ctx.enter_context(tc.tile_pool(name="ct", bufs=2, space="PSUM"))
        sbp = ctx.enter_context(tc.tile_pool(name="sbp", bufs=5))
        sbq = ctx.enter_context(tc.tile_pool(name="sbq", bufs=2))
        sbo = ctx.enter_context(tc.tile_pool(name="sbo", bufs=3))
        sbs = ctx.enter_context(tc.tile_pool(name="sbs", bufs=2))

        # ------------- phase B: KV projection, K rope, V transpose -------
        for nch in range(2):
            pkv = ps.tile([128, 1024], f32, tag="ps", name="pkv")
            for h2 in range(2):
                s2 = slice(nch * 1024 + 512 * h2, nch * 1024 + 512 * (h2 + 1))
                for kt in range(8):
                    nc.tensor.matmul(pkv[:, 512 * h2:512 * (h2 + 1)],
                                     wkv_s[kt][:], xt_s[kt][:, s2],
                                     start=(kt == 0), stop=(kt == 7))
            nc.scalar.copy(kvsb[:, nch * 1024:(nch + 1) * 1024], pkv[:])

        nc.vector.stream_shuffle(ksw[:], kvsb[0:HD, :], swap_mask)
        nc.vector.tensor_mul(kt1[:], kvsb[0:HD, :], ck_s[:])
        nc.vector.tensor_mul(ksw[:], ksw[:], sk_s[:])
        nc.vector.tensor_add(kdup[0:HD, :], kt1[:], ksw[:])
        nc.sync.dma_start(kdup[HD:128, :], kdup[0:HD, :])
        nc.sync.dma_start_transpose(v_big[:, :, 0:HD], kvsb[HD:128, :])

        # ------------- Q projection + rope for one 1024-col chunk --------
        def qproj_chunk(mc, nch):
            sl = slice(nch * 1024, (nch + 1) * 1024)
            pq = ps.tile([128, 1024], f32, tag="ps", name="pq")
            DR = mybir.MatmulPerfMode.DoubleRow
            for h2 in range(2):
                s2 = slice(nch * 1024 + 512 * h2, nch * 1024 + 512 * (h2 + 1))
                for b2 in range(4):
                    nc.tensor.matmul(
                        pq[:, 512 * h2:512 * (h2 + 1)],
                        wq8[:, 2 * b2:2 * b2 + 2, 128 * mc:128 * (mc + 1)],
                        xq8[:, 2 * b2:2 * b2 + 2, s2],
                        start=(b2 == 0), stop=(b2 == 3),
                        perf_mode=DR)
            qraw = sbq.tile([128, 1024], bf16, tag="qraw", name="qraw")
            nc.scalar.copy(qraw[:], pq[:])
            qt1 = sbq.tile([128, 1024], bf16, tag="qt1", name="qt1")
            nc.vector.tensor_mul(qt1[:], qraw[:], cq_s[mc][:, sl])
            qu = sbq.tile([128, 1024], bf16, tag="qu", name="qu")
            nc.vector.tensor_mul(qu[:], qraw[:], sqp_s[mc][:, sl])
            qsw = sbq.tile([128, 1024], bf16, tag="qsw", name="qsw")
            nc.vector.stream_shuffle(qsw[:], qu[:], swap_mask)
            nc.vector.tensor_add(qrope[mc][:, sl], qt1[:], qsw[:])

        qproj_chunk(0, 0)
        qproj_chunk(1, 0)

        # ------------- phase C: pipelined attention stream ---------------
        def attention_qc(qc, inject):
            """inject: list of (due_slot, fn) interleaved into the stream."""
            q0 = qc * 1024
            ctx_t = {}
            pv_done = {}
            pending = []     # (ready_slot, h, kt, pT)
            deferred = []    # (due_slot, seq, fn), kept sorted
            seq = [0]

            def push_deferred(due, fn):
                import bisect
                bisect.insort(deferred, (due, seq[0], fn))
                seq[0] += 1

            slot = [0]
            for due, fn in inject:
                push_deferred(due, fn)

            def emit_pv(h, kt, pT):
                cx = ctx_t[h]
                first = pv_done[h] == 0
                last = pv_done[h] == NKT - 1
                for h2 in range(2):
                    nc.tensor.matmul(cx[:, 512 * h2:512 * (h2 + 1)],
                                     v_t[kt][:],
                                     pT[:, 512 * h2:512 * (h2 + 1)],
                                     start=first, stop=last)
                pv_done[h] += 1
                if last:
                    schedule_norm(h)

            def schedule_norm(h):
                cx = ctx_t[h]
                g = slot[0]

                denr = sbs.tile([1, 1024], f32, tag="denr", name="denr")
                nc.scalar.copy(denr[:], cx[HD:HD + 1, :])
                rcp1_box = []

                def bcast():
                    rcp1 = sbs.tile([1, 1024], f32, tag="rcp1", name="rcp1")
                    nc.vector.reciprocal_approx_fast(rcp1[:], denr[:])
                    rcp = sbs.tile([HD, 1024], f32, tag="rcp", name="rcp")
                    nc.gpsimd.partition_broadcast(rcp[:], rcp1[:])
                    rcp1_box.append(rcp)

                def finish():
                    hb = HD * (h % 2)
                    nc.vector.tensor_mul(
                        ctxn[h // 2][hb:hb + HD, q0:q0 + 1024],
                        cx[0:HD, :], rcp1_box[0][:])

                push_deferred(g + 1, bcast)
                push_deferred(g + 3, finish)

            for h in range(4):
                ctx_t[h] = None
                pv_done[h] = 0
                mcq, hb = h // 2, HD * (h % 2)
                qt = qrope[mcq]
                for kt in range(NKT):
                    g = slot[0]
                    # scores for (h, kt)
                    s = ps.tile([128, 1024], f32, tag="ps", name="s")
                    for h2 in range(2):
                        nc.tensor.matmul(
                            s[:, 512 * h2:512 * (h2 + 1)],
                            kdup[hb:hb + HD, 128 * kt:128 * (kt + 1)],
                            qt[hb:hb + HD, q0 + 512 * h2:q0 + 512 * (h2 + 1)],
                            start=True, stop=True)
                    pT = sbp.tile([128, 1024], bf16, tag="pT", name="pT")
                    if kt in (5, 13):
                        v1 = sbp.tile([128, 1024], bf16, tag="v1", name="v1",
                                      bufs=2)
                        nc.vector.tensor_scalar(
                            v1[:].bitcast(i16), s[:], SCHRA_A,
                            SCHRA_B - 192.0, MUL, ADD)
                        v2 = sbp.tile([128, 1024], bf16, tag="v2", name="v2",
                                      bufs=2)
                        nc.vector.tensor_scalar(
                            v2[:].bitcast(i16), v1[:].bitcast(i16), 64.0,
                            None, ADD)
                        nc.vector.scalar_tensor_tensor(
                            pT[:], v1[:], SQRT2, v2[:], MUL, ADD)
                        ready = g + 4
                    else:
                        nc.scalar.activation(pT[:], s[:], EXP)
                        ready = g + 2
                    if ctx_t[h] is None:
                        ctx_t[h] = ct.tile([128, 1024], f32, tag="ct",
                                           name="cx")
                    pending.append((ready, h, kt, pT))
                    slot[0] += 1
                    # emit due PVs / deferred work
                    while pending and pending[0][0] <= slot[0]:
                        _, ph, pkt, ppT = pending.pop(0)
                        emit_pv(ph, pkt, ppT)
                    while deferred and deferred[0][0] <= slot[0]:
                        deferred.pop(0)[2]()

            # flush
            while pending:
                _, ph, pkt, ppT = pending.pop(0)
                emit_pv(ph, pkt, ppT)
                slot[0] += 1
                while deferred and deferred[0][0] <= slot[0]:
                    deferred.pop(0)[2]()
            slot[0] += 4
            while deferred:
                deferred.pop(0)[2]()

        def d_chunk(qc, mc):
            q0 = qc * 1024
            dp = ps.tile([128, 1024], f32, tag="ps", name="dp")
            for h2 in range(2):
                s2 = slice(q0 + 512 * h2, q0 + 512 * (h2 + 1))
                nc.tensor.matmul(dp[:, 512 * h2:512 * (h2 + 1)],
                                 wo_s[0][:, 128 * mc:128 * (mc + 1)],
                                 ctxn[0][:, s2], start=True, stop=False)
                nc.tensor.matmul(dp[:, 512 * h2:512 * (h2 + 1)],
                                 wo_s[1][:, 128 * mc:128 * (mc + 1)],
                                 ctxn[1][:, s2], start=False, stop=True)
            ob = sbo.tile([128, 1024], bf16, tag="ob", name="ob")
            if mc % 2 == 0:
                nc.vector.tensor_copy(ob[:], dp[:])
            else:
                nc.scalar.copy(ob[:], dp[:])
            nc.sync.dma_start(outT[128 * mc:128 * (mc + 1), q0:q0 + 1024],
                              ob[:])

        # qc0: q-projection for the second column block rides at head
        # boundaries; D(qc0) is interleaved into C(qc1)'s early slots.
        attention_qc(0, [(24, lambda: qproj_chunk(0, 1)),
                         (40, lambda: qproj_chunk(1, 1))])
        attention_qc(1, [(4 + 2 * mc, (lambda m: lambda: d_chunk(0, m))(mc))
                        for mc in range(8)])
        for mc in range(8):
            d_chunk(1, mc)

    nc.compile()
    return nc


def _host_inputs(x, Wq, Wk, Wv, Wo):
    """Build the 8 per-core input maps."""
    bf = ml_dtypes.bfloat16
    inv = 1.0 / (THETA ** (np.arange(0, D, 2, dtype=np.float64) / D))
    t = np.arange(S, dtype=np.float64)
    sgn256 = np.where(np.arange(256) % 2 == 0, -1.0, 1.0)
    sgn64 = sgn256[:HD]
    INVSQ = 1.0 / 32.0   # 1/sqrt(D), folded into the q rope tables
    swap = np.arange(256) ^ 1

    angk = t[None, :] * inv[np.arange(HD) // 2][:, None]
    ck = np.cos(angk).astype(bf)
    sk = (sgn64[:, None] * np.sin(angk)).astype(bf)

    in_maps = []
    for c in range(NCORES):
        b, g = divmod(c, G)
        fq = inv[128 * g + np.arange(256) // 2]
        angq = t[None, :] * fq[:, None]
        sq = INVSQ * sgn256[:, None] * np.sin(angq)
        in_maps.append({
            "xT": np.ascontiguousarray(x[b].T).astype(bf),
            "xq8d": np.ascontiguousarray(np.clip(x[b].T, -240, 240)).astype(
                ml_dtypes.float8_e4m3),
            "wq8d": np.ascontiguousarray(
                Wq[:, 256 * g:256 * (g + 1)]).astype(ml_dtypes.float8_e4m3),
            "wkv": np.ascontiguousarray(np.concatenate(
                [Wk[:, HD * g:HD * (g + 1)],
                 Wv[:, HD * g:HD * (g + 1)]], axis=1)).astype(bf),
            "wo": np.ascontiguousarray(Wo[256 * g:256 * (g + 1), :]).astype(bf),
            "cq": (INVSQ * np.cos(angq)).astype(bf),
            "sqp": np.ascontiguousarray(sq[swap]).astype(bf),
            "ck": ck, "sk": sk,
        })
    return in_maps


def _run(in_maps, trace=False, tmpdir=None):
    global _compiled
    from concourse.bass_utils import run_bass_kernel_spmd
    if _compiled is None:
        _compiled = _build_program()
    return run_bass_kernel_spmd(_compiled, in_maps, list(range(NCORES)),
                                trace=trace, tmpdir=tmpdir)


def kernel(x, Wq, Wk, Wv, Wo, _trace=False, _tmpdir=None):
    x = np.asarray(x, np.float32)
    in_maps = _host_inputs(x, np.asarray(Wq, np.float32),
                           np.asarray(Wk, np.float32),
                           np.asarray(Wv, np.float32),
                           np.asarray(Wo, np.float32))
    res = _run(in_maps, trace=_trace, tmpdir=_tmpdir)
    out = np.zeros((B, S, D), np.float32)
    for c in range(NCORES):
        b = c // G
        out[b] += res.results[c]["outT"].T.astype(np.float32)
    kernel.last_results = res
    return out
